# revision 22
# baseline (speedup 1.0000x reference)
"""APPNP GNN (MLP + 10-hop propagation + log_softmax) on 8 Trainium2 cores.

Strategy
--------
- Nodes are relabeled so core c owns PB = 12544 local slots (98 windows x 128
  dsts); a balanced greedy assignment equalizes per-(window, colgroup, region)
  edge counts across cores to minimize shared-schedule padding.
- The communicated state z' = D^{-1/2} z lives in TWO half-tables split by
  window: half A = windows [0, WA), half B = [WA, 98).  Each hop AllGathers
  the two halves separately; the gather work is likewise split into phase A
  (sources in half A) and phase B.  AG_A overlaps the tail of the current
  hop, AG_B overlaps the next hop's phase-A gathers, so the collectives hide
  almost entirely behind gather DMA.
- Per hop: z'_{k+1} = 0.9*dinv^2 (.) [A z'_k + z'_k] + 0.1*dinv (.) h.
  Phase A accumulates its indicator-matmul partial sums in PSUM and spills
  them to an SBUF f32 buffer (zagg); phase B adds the rest + self loop + h.
- Aggregation: dma_gather (Ant ucode) pulls each edge's source row (256B)
  into SBUF; per-128-slot-block indicator matmuls (fp8 0/1 weights, 32-col
  strips via tile_position) segment-sum into per-window PSUM tiles.
- MLP runs in bf16; staging is one contiguous DMA per half (partition-major
  table rows: row = c*(Wh*128) + p*Wh + w_local).
"""
import os
import sys
import time

sys.path.insert(0, "/opt/trn_rl_repo")
import numpy as np
import ml_dtypes

N = 100000
FIN = 512
HID = 256
C = 64
KHOPS = int(os.environ.get("GNN_HOPS", "10"))
ALPHA = 0.1
NCORES = 8
NW = 98
WA = int(os.environ.get("GNN_WA", "40"))   # windows in half A
WB = NW - WA
PB = NW * 128            # 12544
NTOT = NCORES * PB       # 100352
REGA = 4 * WA * 128      # rows per region in tabA (2 regions)
REGB = 4 * WB * 128      # rows per region in tabB (2 regions)
CW = int(os.environ.get("GNN_CW", "4"))  # windows per chunk
WBATCH = 64              # indicator pieces per W-stream DMA batch
NCHUNKS = (NW + CW - 1) // CW


# ----------------------------------------------------------------------------
# host-side preprocessing
# ----------------------------------------------------------------------------
def _preprocess(x, edge_index):
    t0 = time.time()
    src = np.asarray(edge_index[0], np.int64)
    dst = np.asarray(edge_index[1], np.int64)
    E = src.shape[0]
    assert REGB < 2 ** 15 and REGA < 2 ** 15

    degin = np.bincount(dst, minlength=N)
    deg = degin + 1                                   # + self loop
    dinv = (1.0 / np.sqrt(deg.astype(np.float64))).astype(np.float32)

    g = np.arange(N, dtype=np.int64)
    core_of = g % NCORES

    # region of an edge's source: 2*(half of src window) + (src core >= 4)
    def q_of(loc_arr):
        w_s = loc_arr[src] // 128
        return (np.where(w_s < WA, 0, 2) + (core_of[src] >= 4)).astype(np.int64)

    # balanced within-core bin assignment (greedy argmin-max on per-node
    # in-edge counts split by source region), rank-matched across cores.
    # Two passes: source regions depend on the assignment itself.
    NBINS = NW * 4
    loc_of = g // NCORES
    for _pass in range(2):
        qs = q_of(loc_of)
        vq = np.bincount(dst * 4 + qs, minlength=N * 4).reshape(N, 4)
        new_loc = np.empty(N, np.int64)
        for c in range(NCORES):
            nodes = np.where(core_of == c)[0]
            v = vq[nodes].astype(np.float64)
            order = np.argsort(-v.sum(1), kind="stable")
            nodes, v = nodes[order], v[order]
            binsum = np.zeros((NBINS, 4))
            bincnt = np.zeros(NBINS, np.int64)
            target = v.sum(0) / NBINS
            assign = np.empty(len(nodes), np.int64)
            for i in range(len(nodes)):
                score = (binsum + v[i] - target).max(1)
                score[bincnt >= 32] = 1e18
                b = int(np.argmin(score))
                assign[i] = b
                binsum[b] += v[i]
                bincnt[b] += 1
            key = np.lexsort((binsum[:, 3], binsum[:, 2], binsum[:, 1],
                              binsum[:, 0]))
            rankperm = np.empty(NBINS, np.int64)
            rankperm[key] = np.arange(NBINS)
            fb = rankperm[assign]
            bo = np.lexsort((np.arange(len(nodes)), fb))
            seq2, b2 = nodes[bo], fb[bo]
            within = np.arange(len(seq2)) - np.searchsorted(b2, b2)
            new_loc[seq2] = (b2 // 4) * 128 + (b2 % 4) * 32 + within
        loc_of = new_loc

    # table rows (partition-major within core, per half):
    #   w < WA: rowA = c*WA*128 + p*WA + w
    #   w >= WA: rowB = c*WB*128 + p*WB + (w-WA)
    p_of = loc_of % 128
    w_of = loc_of // 128
    rowh = np.where(
        w_of < WA,
        core_of * (WA * 128) + p_of * WA + w_of,
        core_of * (WB * 128) + p_of * WB + (w_of - WA))

    ndst = core_of[dst] * PB + loc_of[dst]
    q_s = q_of(loc_of)
    srow = rowh[src]
    idx_val_all = np.where(q_s < 2, srow - (q_s % 2) * REGA,
                           srow - (q_s % 2) * REGB).astype(np.int16)

    core_d = (ndst // PB).astype(np.int32)
    li_d = (ndst % PB).astype(np.int32)
    w_d = li_d // 128
    j_d = (li_d % 128) // 32
    ch_d = w_d // CW

    # group = (w, j, q); counts per core
    gidx = (w_d.astype(np.int64) * 4 + j_d) * 4 + q_s
    cnt = np.bincount(core_d.astype(np.int64) * (NW * 16) + gidx,
                      minlength=NCORES * NW * 16).reshape(NCORES, NW, 4, 4)
    cap = cnt.max(axis=0)                             # [NW, 4, 4] shared

    # ---- static shared schedule: phase A (q 0,1) then phase B (q 2,3) ------
    calls = []
    slot_cursor = 0
    group_slot0 = np.zeros((NW, 4, 4), np.int64)
    for phase, qpair in ((0, (0, 1)), (1, (2, 3))):
        for ch in range(NCHUNKS):
            wlist = list(range(ch * CW, min((ch + 1) * CW, NW)))
            for q in qpair:
                c0 = slot_cursor
                groups = []
                for w in wlist:
                    for j in range(4):
                        cp = int(cap[w, j, q])
                        if cp == 0:
                            continue
                        group_slot0[w, j, q] = slot_cursor
                        groups.append((w, j, slot_cursor - c0, cp))
                        slot_cursor += cp
                n_raw = slot_cursor - c0
                n_pad = max(-(-n_raw // 128) * 128, 128)
                slot_cursor = c0 + n_pad
                calls.append(dict(q=q, ch=ch, phase=phase, windows=wlist,
                                  slot0=c0, n=n_pad,
                                  nblk=-(-n_pad // 128), groups=groups))
    S = slot_cursor

    # ---- pieces: runs of (w,j) chopped at 128-slot block edges -------------
    piece_call = []
    piece_blk = []
    piece_w = []
    piece_cb = []
    for ci, cl in enumerate(calls):
        for (w, j, goff, cp) in cl["groups"]:
            a, b = goff, goff + cp
            for blk in range(a // 128, (b - 1) // 128 + 1):
                piece_call.append(ci)
                piece_blk.append(blk)
                piece_w.append(w)
                piece_cb.append(j * 32)
    NP = len(piece_call)
    piece_call = np.array(piece_call, np.int64)
    piece_blk = np.array(piece_blk, np.int64)
    piece_w = np.array(piece_w, np.int64)
    piece_cb = np.array(piece_cb, np.int64)
    piece_phase = np.array([calls[c]["phase"] for c in piece_call], np.int64)
    # stop flag: last piece of each (window, phase)
    piece_stop = np.zeros(NP, bool)
    last_of = {}
    for i in range(NP):
        last_of[(piece_w[i], piece_phase[i])] = i
    for _, i in last_of.items():
        piece_stop[i] = True
    has_phase = np.zeros((NW, 2), bool)
    for (w, ph) in last_of:
        has_phase[w, ph] = True
    assert has_phase.all(), "every window needs pieces in both phases"

    pid_lookup = {}
    for i in range(NP):
        pid_lookup[(piece_call[i], piece_blk[i], piece_w[i],
                    piece_cb[i] // 32)] = i
    slot_piece = np.full(S, -1, np.int64)
    for ci, cl in enumerate(calls):
        c0 = cl["slot0"]
        for (w, j, goff, cp) in cl["groups"]:
            a, b = goff, goff + cp
            for blk in range(a // 128, (b - 1) // 128 + 1):
                lo, hi = max(a, blk * 128), min(b, (blk + 1) * 128)
                pid = pid_lookup[(ci, blk, w, j)]
                slot_piece[c0 + lo: c0 + hi] = pid

    NB = -(-NP // WBATCH)

    # ---- per-core slot assignment (vectorized over edges) ------------------
    phase_d = (q_s >= 2).astype(np.int64)
    perm = np.lexsort((srow, ndst, q_s, ch_d, phase_d, core_d))
    p_core = core_d[perm]
    p_q = q_s[perm]
    p_li = li_d[perm]
    p_w = w_d[perm]
    p_j = j_d[perm]
    p_iv = idx_val_all[perm]
    gkey = ((p_core.astype(np.int64) * NW + p_w) * 4 + p_j) * 4 + p_q
    changes = np.empty(E, bool)
    changes[0] = True
    changes[1:] = gkey[1:] != gkey[:-1]
    gstart = np.maximum.accumulate(np.where(changes, np.arange(E), 0))
    rank = np.arange(E) - gstart
    slot = group_slot0[p_w, p_j, p_q] + rank

    SC = S // 16
    idx_all = np.zeros((NCORES, 16, SC), np.int16)
    f8 = ml_dtypes.float8_e4m3fn
    one_f8 = np.float32(1.0).astype(f8).view(np.uint8)
    Wall = np.zeros((NCORES, NP, 128, 32), np.uint8)
    for c in range(NCORES):
        m = p_core == c
        sl = slot[m]
        arr = np.zeros(S, np.int16)
        arr[sl] = p_iv[m]
        idx_all[c] = arr.reshape(SC, 16).T
        pid = slot_piece[sl]
        assert (pid >= 0).all()
        row = sl % 128
        col = p_li[m] % 32
        flat = Wall[c].reshape(-1)
        flat[(pid * 128 + row) * 32 + col] = one_f8
    idx_dram = np.tile(idx_all, (1, 8, 1))             # [NCORES, 128, SC]

    NPpad = NB * WBATCH
    Wpad = np.zeros((NCORES, NPpad, 128, 32), np.uint8)
    Wpad[:, :NP] = Wall
    Wstream = Wpad.reshape(NCORES, NB, WBATCH, 128, 32) \
                  .transpose(0, 1, 3, 2, 4) \
                  .reshape(NCORES, NB * 128, WBATCH * 32).view(f8)

    # ---- per-core dense inputs --------------------------------------------
    newid = core_of * PB + loc_of
    orig_of_new = np.full(NTOT, -1, np.int64)
    orig_of_new[newid] = g
    xTt = np.zeros((NCORES, NW * 128, FIN), ml_dtypes.bfloat16)
    dinv_t = np.zeros((NCORES, 128, NW), np.float32)
    dsq9_t = np.zeros((NCORES, 128, NW), np.float32)
    sqd_t = np.zeros((NCORES, 128, NW), np.float32)
    x = np.asarray(x, np.float32)
    for c in range(NCORES):
        gids = orig_of_new[c * PB:(c + 1) * PB]
        valid = gids >= 0
        xr = np.zeros((PB, FIN), np.float32)
        xr[valid] = x[gids[valid]]
        xTt[c] = xr.reshape(NW, 128, 4, 128).transpose(0, 3, 2, 1) \
                   .reshape(NW * 128, FIN).astype(ml_dtypes.bfloat16)
        dv = np.where(valid, dinv[np.maximum(gids, 0)], 0).astype(np.float32)
        dinv_t[c] = dv.reshape(NW, 128).T
        dsq9_t[c] = ((1.0 - ALPHA) * dv * dv).astype(np.float32) \
            .reshape(NW, 128).T
        sq = np.where(valid, np.sqrt(deg[np.maximum(gids, 0)])
                      .astype(np.float32), 0)
        sqd_t[c] = sq.astype(np.float32).reshape(NW, 128).T

    sched = dict(calls=calls, NP=NP, NB=NB, S=S, SC=SC,
                 piece_call=piece_call, piece_blk=piece_blk,
                 piece_w=piece_w, piece_cb=piece_cb, piece_stop=piece_stop,
                 piece_phase=piece_phase, orig_of_new=orig_of_new)
    data = dict(idx=idx_dram, W=Wstream, xTt=xTt, dinv=dinv_t,
                dsq9=dsq9_t, sqd=sqd_t)
    print(f"[preprocess] {time.time()-t0:.1f}s  S={S} NP={NP} NB={NB} "
          f"slots/edge={S/E*8:.3f}", flush=True)
    return sched, data


# ----------------------------------------------------------------------------
# device program
# ----------------------------------------------------------------------------
def _build_program(sched):
    from concourse import bacc, mybir, tile, library_config
    from concourse.masks import make_identity

    f32 = mybir.dt.float32
    bf16 = mybir.dt.bfloat16
    fp8 = mybir.dt.float8e4
    i16 = mybir.dt.int16
    AX = mybir.AxisListType
    OP = mybir.AluOpType
    AF = mybir.ActivationFunctionType

    calls = sched["calls"]
    NP, NB, SC = sched["NP"], sched["NB"], sched["SC"]
    pc, pb = sched["piece_call"], sched["piece_blk"]
    pw, pcb, pstop = sched["piece_w"], sched["piece_cb"], sched["piece_stop"]

    nc = bacc.Bacc("TRN2", target_bir_lowering=False, debug=False,
                   num_devices=NCORES)

    xTtT = nc.dram_tensor("xTt", [NW * 128, FIN], bf16, kind="ExternalInput")
    w0T = nc.dram_tensor("w0", [FIN, HID], bf16, kind="ExternalInput")
    b0T = nc.dram_tensor("b0t", [128, 2], f32, kind="ExternalInput")
    w1T = nc.dram_tensor("w1", [HID, C], bf16, kind="ExternalInput")
    b1T = nc.dram_tensor("b1t", [C, 1], f32, kind="ExternalInput")
    dinvT = nc.dram_tensor("dinv", [128, NW], f32, kind="ExternalInput")
    dsq9T = nc.dram_tensor("dsq9", [128, NW], f32, kind="ExternalInput")
    sqdT = nc.dram_tensor("sqd", [128, NW], f32, kind="ExternalInput")
    idxT = nc.dram_tensor("idx", [128, SC], i16, kind="ExternalInput")
    wsT = nc.dram_tensor("ws", [NB * 128, WBATCH * 32], fp8,
                         kind="ExternalInput")
    outT = nc.dram_tensor("out", [PB, C], f32, kind="ExternalOutput")

    stagA = nc.dram_tensor("stagA", [WA * 128 * 128], bf16)
    stagB = nc.dram_tensor("stagB", [WB * 128 * 128], bf16)
    tabsA = [nc.dram_tensor(f"tabA{t}", [NCORES * WA * 128, 128], bf16,
                            addr_space="Shared") for t in range(2)]
    tabsB = [nc.dram_tensor(f"tabB{t}", [NCORES * WB * 128, 128], bf16,
                            addr_space="Shared") for t in range(2)]

    def emit_ag(stag_t, dst_tab):
        nc.gpsimd.collective_compute(
            "AllGather", OP.bypass,
            replica_groups=[list(range(NCORES))],
            ins=[stag_t.ap().opt()], outs=[dst_tab.ap().opt()],
        )

    with tile.TileContext(nc) as tc:
        with tc.tile_pool(name="const", bufs=1) as cpool, \
             tc.tile_pool(name="state", bufs=1) as spool, \
             tc.tile_pool(name="msg", bufs=6) as mpool, \
             tc.tile_pool(name="wbuf", bufs=3) as wpool, \
             tc.tile_pool(name="ibuf", bufs=6) as ipool, \
             tc.tile_pool(name="work", bufs=4) as tpool:

            nc.gpsimd.load_library(library_config.mlp)

            w0sb = cpool.tile([128, 4 * HID], bf16)
            for k in range(4):
                nc.sync.dma_start(out=w0sb[:, k * HID:(k + 1) * HID],
                                  in_=w0T[k * 128:(k + 1) * 128, :])
            w1sb = cpool.tile([128, 2 * C], bf16)
            for k in range(2):
                nc.sync.dma_start(out=w1sb[:, k * C:(k + 1) * C],
                                  in_=w1T[k * 128:(k + 1) * 128, :])
            b0sb = cpool.tile([128, 2], f32)
            nc.sync.dma_start(out=b0sb[:, :], in_=b0T[:, :])
            b1sb = cpool.tile([C, 1], f32)
            nc.sync.dma_start(out=b1sb[:, :], in_=b1T[:, :])
            dinvsb = cpool.tile([128, NW], f32)
            nc.sync.dma_start(out=dinvsb[:, :], in_=dinvT[:, :])
            dsq9sb = cpool.tile([128, NW], f32)
            nc.sync.dma_start(out=dsq9sb[:, :], in_=dsq9T[:, :])
            sqdsb = cpool.tile([128, NW], f32)
            nc.sync.dma_start(out=sqdsb[:, :], in_=sqdT[:, :])
            idsb = cpool.tile([128, 128], f32)
            make_identity(nc, idsb[:, :])
            zcov = cpool.tile([128, 128], bf16)
            nc.vector.memset(zcov[:, :], 0.0)

            zf32 = spool.tile([128, NW * C], f32)     # resident f32 state z'
            hpp = spool.tile([128, NW * C], f32)      # 0.1 * dinv (.) h
            zagg = spool.tile([128, NW * C], f32)     # phase-A partial aggs
            st = spool.tile([128, NW * 128], bf16)    # staged bf16 state

            def stage_window(w, zsl):
                nc.vector.tensor_copy(out=st[:, w * 128:w * 128 + C], in_=zsl)

            def flush_stage(half):
                if half == 0:
                    nc.sync.dma_start(
                        out=stagA[:].rearrange("(p e) -> p e", p=128),
                        in_=st[:, 0:WA * 128])
                else:
                    nc.sync.dma_start(
                        out=stagB[:].rearrange("(p e) -> p e", p=128),
                        in_=st[:, WA * 128:NW * 128])

            # ---------------- MLP + initial state ----------------
            with tc.tile_pool(name="mx", bufs=3) as xpool, \
                 tc.tile_pool(name="mh", bufs=2) as hpool, \
                 tc.tile_pool(name="mh2", bufs=2) as h2pool, \
                 tc.tile_pool(name="mps", bufs=2, space="PSUM") as mpsp:
                for w in range(NW):
                    xt = xpool.tile([128, FIN], bf16)
                    nc.sync.dma_start(out=xt[:, :],
                                      in_=xTtT[w * 128:(w + 1) * 128, :])
                    ph = mpsp.tile([128, 256], f32, space="PSUM")
                    for hh in range(2):
                        for k in range(4):
                            nc.tensor.matmul(
                                out=ph[:, hh * 128:(hh + 1) * 128],
                                lhsT=w0sb[:, k * HID + hh * 128:
                                          k * HID + (hh + 1) * 128],
                                rhs=xt[:, k * 128:(k + 1) * 128],
                                start=(k == 0), stop=(k == 3))
                    hT = hpool.tile([128, 256], bf16)
                    for hh in range(2):
                        nc.scalar.activation(
                            out=hT[:, hh * 128:(hh + 1) * 128],
                            in_=ph[:, hh * 128:(hh + 1) * 128],
                            func=AF.Relu, bias=b0sb[:, hh:hh + 1])
                    ps2 = mpsp.tile([C, 128], f32, space="PSUM")
                    for kk in range(2):
                        nc.tensor.matmul(out=ps2[:, :],
                                         lhsT=w1sb[:, kk * C:(kk + 1) * C],
                                         rhs=hT[:, kk * 128:(kk + 1) * 128],
                                         start=(kk == 0), stop=(kk == 1))
                    h2T = h2pool.tile([C, 128], f32)
                    nc.scalar.activation(out=h2T[:, :], in_=ps2[:, :],
                                         func=AF.Identity, bias=b1sb[:, 0:1])
                    ps3 = mpsp.tile([128, C], f32, space="PSUM")
                    nc.tensor.transpose(out=ps3[:, :], in_=h2T[:, :],
                                        identity=idsb[0:C, 0:C])
                    zsl = zf32[:, w * C:(w + 1) * C]
                    nc.vector.tensor_scalar(out=zsl, in0=ps3[:, :],
                                            scalar1=dinvsb[:, w:w + 1],
                                            scalar2=None, op0=OP.mult)
                    nc.vector.tensor_scalar(out=hpp[:, w * C:(w + 1) * C],
                                            in0=zsl, scalar1=ALPHA,
                                            scalar2=None, op0=OP.mult)
                    stage_window(w, zsl)
                    if w == WA - 1:
                        flush_stage(0)
                        emit_ag(stagA, tabsA[0])
                flush_stage(1)
            emit_ag(stagB, tabsB[0])

            # ---------------- propagation hops ----------------
            callsA = [cl for cl in calls if cl["phase"] == 0]
            callsB = [cl for cl in calls if cl["phase"] == 1]
            NPA = int((sched["piece_phase"] == 0).sum())

            for k in range(KHOPS):
                tabA_in = tabsA[k % 2]
                tabB_in = tabsB[k % 2]
                last = (k == KHOPS - 1)

                def gather_calls(chcalls, tab_in, reg):
                    # one idx DMA for the chunk's calls (contiguous columns)
                    col0 = chcalls[0]["slot0"] // 16
                    colN = (chcalls[-1]["slot0"] + chcalls[-1]["n"]) // 16
                    it = ipool.tile([128, colN - col0], i16, name="it")
                    nc.sync.dma_start(out=it[:, :],
                                      in_=idxT[:, col0:colN])
                    mts = {}
                    for cl in chcalls:
                        off = cl["slot0"] // 16 - col0
                        ncols = cl["n"] // 16
                        mt = mpool.tile([128, cl["nblk"] * 128], bf16,
                                        name="mt")
                        qq = cl["q"] % 2
                        nc.gpsimd.dma_gather(
                            out_ap=mt[:, :].rearrange(
                                "p (b e) -> p b e", e=128),
                            in_ap=tab_in[qq * reg:(qq + 1) * reg, :],
                            idxs_ap=it[:, off:off + ncols],
                            num_idxs=cl["n"], num_idxs_reg=cl["n"],
                            elem_size=128, single_packet=False)
                        mts[cl["q"]] = mt
                    return mts

                wtile = [None]

                def do_pieces(pi, pi_end, mtiles, ptiles):
                    while pi < pi_end:
                        if pi % WBATCH == 0:
                            wtile[0] = wpool.tile([128, WBATCH * 32], fp8, name="wt")
                            b = pi // WBATCH
                            nc.scalar.dma_start(
                                out=wtile[0][:, :],
                                in_=wsT[b * 128:(b + 1) * 128, :])
                        cl = calls[pc[pi]]
                        mt = mtiles[cl["q"]]
                        cb = int(pcb[pi])
                        nc.tensor.matmul(
                            out=ptiles[int(pw[pi])][cb:cb + 32, :],
                            lhsT=wtile[0][:, (pi % WBATCH) * 32:
                                          (pi % WBATCH + 1) * 32],
                            rhs=mt[:, int(pb[pi]) * 128:
                                   int(pb[pi]) * 128 + C],
                            start=False, stop=bool(pstop[pi]),
                            tile_position=(0, cb))
                        pi += 1
                    return pi

                with tc.tile_pool(name=f"ps{k}", bufs=6, space="PSUM") as psp:
                    # ---------- phase A: sources in half A ----------
                    pi = 0
                    for ch in range(NCHUNKS):
                        chcalls = [cl for cl in callsA if cl["ch"] == ch]
                        mtiles = gather_calls(chcalls, tabA_in, REGA)
                        wlist = chcalls[0]["windows"]
                        pbank = psp.tile([128, CW * C], f32, space="PSUM",
                                         name="pbank")
                        ptiles = {}
                        for w in wlist:
                            pt = pbank[:, (w % CW) * C:(w % CW + 1) * C]
                            nc.tensor.matmul(out=pt, lhsT=zcov[:, :],
                                             rhs=zcov[:, 0:C],
                                             start=True, stop=False)
                            ptiles[w] = pt
                        pi_end = pi
                        while pi_end < NPA and calls[pc[pi_end]]["ch"] == ch:
                            pi_end += 1
                        pi = do_pieces(pi, pi_end, mtiles, ptiles)
                        for w in wlist:
                            nc.vector.tensor_copy(
                                out=zagg[:, w * C:(w + 1) * C],
                                in_=ptiles[w])
                    # ---------- phase B: sources in half B ----------
                    pi = NPA
                    for ch in range(NCHUNKS):
                        chcalls = [cl for cl in callsB if cl["ch"] == ch]
                        mtiles = gather_calls(chcalls, tabB_in, REGB)
                        wlist = chcalls[0]["windows"]
                        pbank = psp.tile([128, CW * C], f32, space="PSUM",
                                         name="pbank")
                        ptiles = {}
                        for w in wlist:
                            pt = pbank[:, (w % CW) * C:(w % CW + 1) * C]
                            nc.tensor.matmul(out=pt, lhsT=zcov[:, :],
                                             rhs=zcov[:, 0:C],
                                             start=True, stop=False)
                            ptiles[w] = pt
                        pi_end = pi
                        while pi_end < NP and calls[pc[pi_end]]["ch"] == ch:
                            pi_end += 1
                        pi = do_pieces(pi, pi_end, mtiles, ptiles)
                        for w in wlist:
                            zsl = zf32[:, w * C:(w + 1) * C]
                            t1 = tpool.tile([128, C], f32)
                            nc.vector.tensor_tensor(
                                out=t1[:, :], in0=ptiles[w],
                                in1=zagg[:, w * C:(w + 1) * C], op=OP.add)
                            nc.vector.tensor_tensor(out=t1[:, :], in0=t1[:, :],
                                                    in1=zsl, op=OP.add)
                            nc.vector.tensor_scalar(
                                out=t1[:, :], in0=t1[:, :],
                                scalar1=dsq9sb[:, w:w + 1], scalar2=None,
                                op0=OP.mult)
                            if not last:
                                nc.vector.tensor_tensor(
                                    out=zsl, in0=t1[:, :],
                                    in1=hpp[:, w * C:(w + 1) * C], op=OP.add)
                                stage_window(w, zsl)
                                if w == WA - 1:
                                    flush_stage(0)
                                    emit_ag(stagA, tabsA[(k + 1) % 2])
                                elif w == NW - 1:
                                    flush_stage(1)
                            else:
                                nc.vector.tensor_tensor(
                                    out=t1[:, :], in0=t1[:, :],
                                    in1=hpp[:, w * C:(w + 1) * C], op=OP.add)
                                nc.vector.tensor_scalar(
                                    out=t1[:, :], in0=t1[:, :],
                                    scalar1=sqdsb[:, w:w + 1], scalar2=None,
                                    op0=OP.mult)
                                mx = tpool.tile([128, 1], f32)
                                nc.vector.tensor_reduce(
                                    out=mx[:, :], in_=t1[:, :], axis=AX.X,
                                    op=OP.max)
                                nmx = tpool.tile([128, 1], f32)
                                nc.vector.tensor_scalar(
                                    out=nmx[:, :], in0=mx[:, :], scalar1=-1.0,
                                    scalar2=None, op0=OP.mult)
                                ex = tpool.tile([128, C], f32)
                                se = tpool.tile([128, 1], f32)
                                nc.scalar.activation(
                                    out=ex[:, :], in_=t1[:, :], func=AF.Exp,
                                    bias=nmx[:, 0:1], accum_out=se[:, 0:1])
                                lse = tpool.tile([128, 1], f32)
                                nc.scalar.activation(out=lse[:, :],
                                                     in_=se[:, :], func=AF.Ln)
                                nc.vector.tensor_tensor(
                                    out=mx[:, :], in0=mx[:, :], in1=lse[:, :],
                                    op=OP.add)
                                ot = tpool.tile([128, C], f32)
                                nc.vector.tensor_scalar(
                                    out=ot[:, :], in0=t1[:, :],
                                    scalar1=mx[:, 0:1], scalar2=None,
                                    op0=OP.subtract)
                                nc.sync.dma_start(
                                    out=outT[w * 128:(w + 1) * 128, :],
                                    in_=ot[:, :])
                if not last:
                    emit_ag(stagB, tabsB[(k + 1) % 2])

    t0 = time.time()
    nc.compile()
    print(f"[compile] bacc compile {time.time()-t0:.1f}s", flush=True)
    return nc


# ----------------------------------------------------------------------------
# entry point
# ----------------------------------------------------------------------------
_LAST_NC = None


def _run(inputs, trace=False):
    global _LAST_NC
    from concourse.bass_utils import run_bass_kernel_spmd

    x = np.asarray(inputs["x"], np.float32)
    w0 = np.asarray(inputs["w0"], np.float32)
    b0 = np.asarray(inputs["b0"], np.float32)
    w1 = np.asarray(inputs["w1"], np.float32)
    b1 = np.asarray(inputs["b1"], np.float32)
    edge_index = np.asarray(inputs["edge_index"])

    sched, data = _preprocess(x, edge_index)
    t0 = time.time()
    nc = _build_program(sched)
    _LAST_NC = nc
    print(f"[build+compile] total {time.time()-t0:.1f}s", flush=True)

    b0t = b0.reshape(2, 128).T.copy()
    b1c = b1.reshape(C, 1).copy()
    w0b = w0.astype(ml_dtypes.bfloat16)
    w1b = w1.astype(ml_dtypes.bfloat16)
    in_maps = []
    for c in range(NCORES):
        in_maps.append({
            "xTt": data["xTt"][c],
            "w0": w0b, "b0t": b0t, "w1": w1b, "b1t": b1c,
            "dinv": data["dinv"][c], "dsq9": data["dsq9"][c],
            "sqd": data["sqd"][c],
            "idx": data["idx"][c], "ws": data["W"][c],
        })
    t0 = time.time()
    res = run_bass_kernel_spmd(nc, in_maps, core_ids=list(range(NCORES)),
                               trace=trace)
    print(f"[run] {time.time()-t0:.1f}s exec_time_ns={res.exec_time_ns}",
          flush=True)

    out = np.empty((N, C), np.float32)
    oon = sched["orig_of_new"]
    for c in range(NCORES):
        gids = oon[c * PB:(c + 1) * PB]
        valid = gids >= 0
        out[gids[valid]] = res.results[c]["out"][np.where(valid)[0]]
    return out, res


def kernel(**inputs):
    out, _ = _run(inputs, trace=False)
    return out
